# revision 7
# baseline (speedup 1.0000x reference)
"""Trainium2 Bass kernel: batched single-head self-attention.

Reference computation (per (b, l) pair, 20 independent blocks):
    X = x[b, l] viewed as [N=1024, D=256] (xf layout)
    out[b, l] = softmax(beta * X @ X.T, axis=-1) @ X

Key observations driving the implementation:
  * The HBM layout of x[b, l] is [D, N] == X^T, which is exactly the
    layout the TensorEngine wants for computing scores S = X @ X.T
    (contraction over D on partitions).
  * S is symmetric, so a PSUM tile of S can be read as [keys, queries]
    with no transpose. The softmax shift c_n (subtracted per QUERY, i.e.
    along the free axis) rides the score matmul itself as an extra K=1
    accumulation term (lhsT = ones row, rhs = -c row), so
    W[m, n] = exp(beta * (S[m, n] - c_n)) comes out of a single ScalarE
    activation pass with no bias tensor and no transposes.
  * c_n = ||x_n||^2 (the score diagonal) is a valid softmax shift here
    (exp arguments stay <= ~0) and is computed with a ones-vector matmul
    over X^2 -- no max reduction needed.
  * The second matmul O = P @ X needs X in [keys, D] layout: produced
    on-chip with 16 TensorE transposes per block. A ones column appended
    to that operand makes the softmax denominator Z_n fall out of the
    same matmul, already in per-partition layout for the final scale.

Sharding: 20 blocks over 8 cores as 2 full blocks + 1 half block (512
queries) per core -- exact, no padded compute. The half blocks use a
host-side rotation of the key axis so every core runs the identical
program (softmax is invariant to key permutation when values are
permuted identically).

float32r: relaxed-precision fp32 matmul mode -- the PE streams 1 col/cycle
(for free dim >= 256) instead of 4 cyc/col for exact fp32. The BIR
verifier requires every producer feeding an fp32r matmul to emit fp32r,
so the whole operand chain (HBM input included) is declared fp32r when
fast=True; numpy sees both as float32.
"""

import numpy as np

import concourse.tile as tile
from concourse import bacc, mybir
from concourse.bass_utils import run_bass_kernel_spmd
from concourse.masks import make_identity

F32 = mybir.dt.float32
F32R = mybir.dt.float32r

B, L, D, H, W = 4, 5, 256, 32, 32
N = H * W            # 1024 keys per block
NBLK = B * L         # 20
NCORES = 8
NFULL = 2            # full blocks per core
NSLAB = 3            # 2 full + 1 half
OUT_ROWS = NFULL * N + N // 2  # 2560 rows per core

EXP = mybir.ActivationFunctionType.Exp


def build_program(beta: float, fast: bool = True):
    mdt = F32R if fast else F32
    nc = bacc.Bacc("TRN2", target_bir_lowering=False, debug=False,
                   num_devices=NCORES)
    x_in = nc.dram_tensor("x_in", [NSLAB, D, N], mdt, kind="ExternalInput")
    y_out = nc.dram_tensor("y_out", [OUT_ROWS, D], F32, kind="ExternalOutput")

    with tile.TileContext(nc) as tc:
        _build(tc, nc, x_in.ap(), y_out.ap(), beta, mdt)
    nc.finalize()
    return nc


def _build(tc, nc, x_in, y_out, beta, mdt):
    import contextlib
    ctx = contextlib.ExitStack()
    with ctx:
        const = ctx.enter_context(tc.tile_pool(name="const", bufs=1))
        xt_pool = ctx.enter_context(tc.tile_pool(name="xt", bufs=2))
        xsq_pool = ctx.enter_context(tc.tile_pool(name="xsq", bufs=2))
        w_pool = ctx.enter_context(tc.tile_pool(name="w", bufs=10))
        xfo_pool = ctx.enter_context(tc.tile_pool(name="xfo", bufs=10))
        negc_pool = ctx.enter_context(tc.tile_pool(name="negc", bufs=2))
        osb_pool = ctx.enter_context(tc.tile_pool(name="osb", bufs=2))
        r_pool = ctx.enter_context(tc.tile_pool(name="rcp", bufs=4))
        # PSUM budget: 4 + 2 + 1 + 1 = 8 banks exactly.
        ps_s = ctx.enter_context(tc.tile_pool(name="ps_s", bufs=2, space="PSUM"))
        ps_o = ctx.enter_context(tc.tile_pool(name="ps_o", bufs=2, space="PSUM"))
        ps_t = ctx.enter_context(tc.tile_pool(name="ps_t", bufs=1, space="PSUM"))
        ps_c = ctx.enter_context(tc.tile_pool(name="ps_c", bufs=1, space="PSUM"))

        # Constants. memset/affine_select cannot emit float32r, so build
        # them in fp32 scratch and round via a DVE cast-copy.
        ident_f32 = const.tile([128, 128], F32)
        make_identity(nc, ident_f32[:])
        ones_row_f32 = const.tile([1, 128], F32)
        nc.gpsimd.memset(ones_row_f32[:], 1.0)
        # [1, 0] column pair: appended to the value operand (fp32r needs
        # even free counts, so the ones column gets a zero sibling).
        onezero_f32 = const.tile([128, 2], F32)
        nc.gpsimd.memset(onezero_f32[:, 0:1], 1.0)
        nc.gpsimd.memset(onezero_f32[:, 1:2], 0.0)
        # fp32r also needs even free counts on the stationary operand, so
        # the ones-vector for the norm reduction is [128, 2] (M=2, row 0 read).
        neg_ones2_f32 = const.tile([128, 2], F32)
        nc.gpsimd.memset(neg_ones2_f32[:], -1.0)
        if mdt is F32:
            ident, ones_row = ident_f32, ones_row_f32
            neg_ones2, onezero = neg_ones2_f32, onezero_f32
        else:
            ident = const.tile([128, 128], mdt)
            nc.vector.tensor_copy(ident[:], ident_f32[:])
            ones_row = const.tile([1, 128], mdt)
            nc.vector.tensor_copy(ones_row[:], ones_row_f32[:])
            neg_ones2 = const.tile([128, 2], mdt)
            nc.vector.tensor_copy(neg_ones2[:], neg_ones2_f32[:])
            onezero = const.tile([128, 2], mdt)
            nc.vector.tensor_copy(onezero[:], onezero_f32[:])

        for s in range(NSLAB):
            n_q = N if s < NFULL else N // 2
            n_qt = n_q // 128   # query tiles of 128
            n_h = n_q // 512    # PSUM bank halves

            # X^T slab: [128 partitions, 2 d-chunks, 1024 keys]
            xt = xt_pool.tile([128, 2, N], mdt, tag="xt")
            nc.sync.dma_start(
                out=xt[:], in_=x_in[s].rearrange("(c p) n -> p c n", c=2))

            # x^2 over query columns (for the diagonal-shift row c_n)
            xsq = xsq_pool.tile([128, 2, N], mdt, tag="xsq")
            for c in range(2):
                nc.vector.tensor_mul(xsq[:, c, :n_q], xt[:, c, :n_q],
                                     xt[:, c, :n_q])

            # negc_row[0, n] = -||x_n||^2  via ones-matmul over xsq
            negc = negc_pool.tile([1, N], mdt, tag="negc")
            for h in range(n_h):
                hs = slice(h * 512, (h + 1) * 512)
                cps = ps_c.tile([2, 512], F32, tag="cps")
                nc.tensor.matmul(cps[:], neg_ones2[:], xsq[:, 0, hs],
                                 start=True, stop=False)
                nc.tensor.matmul(cps[:], neg_ones2[:], xsq[:, 1, hs],
                                 start=False, stop=True)
                nc.vector.tensor_copy(negc[:, hs], cps[0:1, :])

            # Value operand: xfo[t] = [X[t-tile, :] | 1]  (keys on partitions)
            xfo_tiles = []
            for t in range(8):
                tps = ps_t.tile([128, 256], mdt, tag="tps")
                for c in range(2):
                    nc.tensor.transpose(tps[:, c * 128:(c + 1) * 128],
                                        xt[:, c, t * 128:(t + 1) * 128],
                                        ident[:])
                xfo = xfo_pool.tile([128, 258], mdt, tag="xfo")
                nc.vector.tensor_copy(xfo[:, 0:256], tps[:])
                nc.vector.tensor_copy(xfo[:, 256:258], onezero[:])
                xfo_tiles.append(xfo)

            # Shifted scores S'[m, n] = S[m, n] - c_n, then W = exp(beta*S')
            w_tiles = []
            for a in range(8):      # key tile (partitions of S')
                sps = ps_s.tile([128, N], F32, tag="sps")
                asl = slice(a * 128, (a + 1) * 128)
                for h in range(n_h):
                    hs = slice(h * 512, (h + 1) * 512)
                    nc.tensor.matmul(sps[:, hs], xt[:, 0, asl], xt[:, 0, hs],
                                     start=True, stop=False)
                    nc.tensor.matmul(sps[:, hs], xt[:, 1, asl], xt[:, 1, hs],
                                     start=False, stop=False)
                    nc.tensor.matmul(sps[:, hs], ones_row[:], negc[:, hs],
                                     start=False, stop=True)
                wt = w_pool.tile([128, N], mdt, tag="w")
                nc.scalar.activation(wt[:, :n_q], sps[:, :n_q], EXP,
                                     scale=float(beta))
                w_tiles.append(wt)

            # O_unnorm[n, :] = sum_m W[m, n] * [x_m | 1 | 0]; col 256 = Z_n
            osb = osb_pool.tile([128, 8, 256], F32, tag="osb")
            for t in range(n_qt):
                ops = ps_o.tile([128, 258], F32, tag="ops")
                tsl = slice(t * 128, (t + 1) * 128)
                for a in range(8):
                    nc.tensor.matmul(ops[:], w_tiles[a][:, tsl],
                                     xfo_tiles[a][:],
                                     start=(a == 0), stop=(a == 7))
                rcp = r_pool.tile([128, 1], F32, tag="rcp")
                nc.vector.reciprocal(rcp[:], ops[:, 256:257])
                nc.vector.tensor_scalar_mul(osb[:, t, :], ops[:, 0:256],
                                            rcp[:])

            nc.sync.dma_start(
                out=y_out[s * N: s * N + n_q].rearrange(
                    "(t p) d -> p t d", t=n_qt),
                in_=osb[:, :n_qt, :])


_PROG_CACHE = {}


def _get_program(beta: float, fast: bool = True):
    key = (beta, fast)
    if key not in _PROG_CACHE:
        _PROG_CACHE[key] = build_program(beta, fast)
    return _PROG_CACHE[key]


def make_in_maps(x: np.ndarray):
    """Shard the full input [B, L, D, H, W] into 8 per-core input maps."""
    xt_all = np.ascontiguousarray(x.reshape(NBLK, D, N))
    in_maps = []
    for c in range(NCORES):
        half_blk = NFULL * NCORES + c // 2
        half = xt_all[half_blk]
        if c % 2 == 1:
            # rotate keys so this core's queries are columns 0..511
            half = np.concatenate([half[:, N // 2:], half[:, :N // 2]], axis=1)
        slab = np.stack([xt_all[NFULL * c], xt_all[NFULL * c + 1], half])
        in_maps.append({"x_in": np.ascontiguousarray(slab, dtype=np.float32)})
    return in_maps


def assemble_output(results):
    """Gather 8 per-core [2560, 256] outputs into [B, L, N, D]."""
    out = np.empty((NBLK, N, D), np.float32)
    for c in range(NCORES):
        y = results[c]["y_out"]
        out[NFULL * c] = y[0:N]
        out[NFULL * c + 1] = y[N:2 * N]
        half_blk = NFULL * NCORES + c // 2
        lo = (c % 2) * (N // 2)
        out[half_blk, lo:lo + N // 2] = y[2 * N:OUT_ROWS]
    return out.reshape(B, L, N, D)


def kernel(x, beta, _trace=False, _fast=True):
    x = np.asarray(x, dtype=np.float32)
    assert x.shape == (B, L, D, H, W), x.shape
    beta_f = float(np.asarray(beta))
    prog = _get_program(beta_f, _fast)
    in_maps = make_in_maps(x)
    res = run_bass_kernel_spmd(prog, in_maps, core_ids=list(range(NCORES)),
                               trace=_trace)
    out = assemble_output(res.results)
    if _trace:
        return out, res
    return out
